# revision 29
# baseline (speedup 1.0000x reference)
"""Trainium2 Bass kernel for the DGL-JTNN tree decoder (nn_DGLJTNNDecoder).

Data-parallel over the 512 trees, 64 trees per NeuronCore. v2 redesign:
  - E-tables (Ez/Eh/Er from emb@W folds) are materialized PER STEP on the
    host into a contiguous slab, streamed with plain sync DMA (no per-step
    dma_gather descriptor generation on the Q7).
  - State log (m|rm, one 2KB row per tree per step) gathered COMPACTLY:
    at most 128 rows per step (only lag>=2 contributions exist there thanks
    to DFS lag parity: pred lags are odd, node lags even), row-major
    no-transpose gather + one-hot routing matmuls on the tensor engine
    that sum the rows into per-tree feature-major S/ARM/HP PSUM tiles.
  - lag-1 pred contributions via masked adds from the SBUF-resident M tile.
  - E contributions accumulated into the z/h and r PSUM groups via identity
    matmuls before the gate matmuls (off the critical chain).
  - Heads unchanged except Eu rows come from a host-built slab (plain DMA)
    and exp runs without the max bias (logits are small).
Losses/accuracies reduce to 8 partial sums per core, combined on host.
"""

import numpy as np
import ml_dtypes

import concourse.bass as bass
import concourse.bacc as bacc
import concourse.mybir as mybir
import concourse.tile as tile
from concourse.library_config import mlp as _mlp_lib
from concourse.bass_utils import run_bass_kernel_spmd

f16 = mybir.dt.float16
f32 = mybir.dt.float32
i16 = mybir.dt.int16
AF = mybir.ActivationFunctionType
ALU = mybir.AluOpType

# problem constants (hardcoded per contract)
B, N, H, L, V = 512, 20, 450, 56, 780
T = 2 * (N - 1)            # 38 steps
NC = 8                     # cores
C = B // NC                # 64 trees/core
Hp = 512                   # padded hidden
NBLK = 40                  # head col blocks (39 real + 1 pad) -> 2560 cols
NCOL = NBLK * C            # 2560
RC = NCOL // 128           # 20 row chunks
NIdx = 128                 # compact gather rows per step
LOG_ROWS = T * 64          # state log rows (tree-major, 2KB each)

import os


def _wrap_idx(idx):
    """[n*16] flat gather order -> [16, n] wrapped, replicated to 128 rows."""
    idx = np.asarray(idx, np.int16)
    n = idx.shape[0] // 16
    return np.tile(idx.reshape(n, 16).T, (8, 1))    # [128, n]


def _host_prep(inputs):
    inp = {k: np.asarray(v) for k, v in inputs.items()}
    (tree_vec, emb, Wz, bz, Wh, bh, Wr, Ur, br, Ww, bw, Uw, bu, Wo, bo,
     Us, bs) = (inp[k] for k in
                ['tree_vec', 'emb', 'Wz', 'bz', 'Wh', 'bh', 'Wr', 'Ur', 'br',
                 'Ww', 'bw', 'Uw', 'bu', 'Wo', 'bo', 'Us', 'bs'])
    wid, root_ids = inp['wid'], inp['root_ids']
    edge_src, edge_dst = inp['edge_src'], inp['edge_dst']
    edge_pred, node_in = inp['edge_pred'], inp['node_in']
    step_eid, step_v = inp['step_eid'], inp['step_v']
    q_rows, q_tgt, p_tgt = inp['q_rows'], inp['q_tgt'], inp['p_tgt']
    n_edges = edge_src.shape[0]
    P, Dn = edge_pred.shape[1], node_in.shape[1]

    def padHp(M, axis):
        pads = [(0, 0)] * M.ndim
        pads[axis] = (0, Hp - M.shape[axis])
        return np.pad(M, pads)

    def h16(x):
        return np.ascontiguousarray(x.astype(np.float16))

    # ---- folded tables (fp32 math) ----
    Ez = padHp(emb @ Wz[:H] + bz, 1)          # [V, Hp]
    Eh = padHp(emb @ Wh[:H] + bh, 1)
    Er = padHp(emb @ Wr + br, 1)
    Eu = padHp(emb @ Uw[:H] + bu, 1)
    Eu[:, 511] = 1.0                          # bias-injection col (bs via us row)

    def fmajor(tbl, rows):                    # rows [T, C] -> [T, 128, 4, C]
        g = tbl[rows]                         # [T, C, Hp]
        return g.transpose(0, 2, 1).reshape(T, 4, 128, rows.shape[1]) \
                .transpose(0, 2, 1, 3)

    def packW(Wm):                            # [512,512] -> [128, 4*512]
        return h16(Wm.reshape(4, 128, Hp).transpose(1, 0, 2).reshape(128, 4 * Hp))

    wz_h = packW(padHp(padHp(Wz[H:], 0), 1))
    wh_h = packW(padHp(padHp(Wh[H:], 0), 1))
    wu_h = packW(padHp(padHp(Ur, 0), 1))
    wuwh_h = packW(padHp(padHp(Uw[H:2 * H], 0), 1))
    wwwh_h = packW(padHp(padHp(Ww[:H], 0), 1))
    Wop = padHp(Wo, 0)
    Wop[511, :] = bo                          # bias row (qrelu[511]==1)
    wo_h = h16(Wop.reshape(4, 128, V).transpose(1, 0, 2).reshape(128, 4 * V))
    Usp = padHp(Us, 0)
    Usp[511, 0] = bs[0]                       # bias row (prelu[511]==1)
    us_h = h16(Usp.reshape(4, 128).T)         # [128, 4]

    # tree_vec with ones column (bias channel), fp32 math for tun/twn
    tvpad = np.zeros((B, 64), np.float32)
    tvpad[:, :L] = tree_vec
    tvpad[:, L] = 1.0
    WuL = padHp(Uw[2 * H:], 1)                # [L, Hp]
    WuLx = np.zeros((64, Hp), np.float32)
    WuLx[:L] = WuL
    WwLx = np.zeros((64, Hp), np.float32)
    WwLx[:L] = padHp(Ww[H:], 1)
    WwLx[L, :H] = bw                          # ones channel -> +bw
    WwLx[L, 511] = 1.0                        # makes qrelu[:,511]=1 (bias row)

    # step scheduling metadata
    estep = np.full(n_edges, -1, np.int64)
    for t in range(T):
        for b in range(B):
            estep[step_eid[t, b]] = t

    cores = []
    for core in range(NC):
        trees = np.arange(core * C, (core + 1) * C)

        # ---- compact gather rows + one-hot routing + lag-1 masks ----
        sidx = np.full((T, NIdx), -1, np.int16)
        atab = np.zeros((T, NIdx, 192), np.float16)   # A_s | A_r | A_n
        masks1 = np.zeros((T, 128), np.float16)
        for t in range(T):
            slot_of = {}
            def slot(row):
                s = slot_of.get(row)
                if s is None:
                    s = len(slot_of)
                    assert s < NIdx
                    slot_of[row] = s
                    sidx[t, s] = row
                return s
            for j in range(C):
                b = trees[j]
                e = step_eid[t, b]
                v = step_v[t, b]
                for p in range(P):
                    pe = edge_pred[e, p]
                    if pe >= n_edges:
                        continue
                    tp = estep[pe]
                    assert tp != t
                    if tp > t:
                        continue
                    if tp == t - 1:
                        masks1[t, j] = 1.0
                        masks1[t, 64 + j] = 1.0
                    else:
                        s = slot(tp * 64 + j)
                        atab[t, s, j] += 1.0          # m -> S
                        atab[t, s, 64 + j] += 1.0     # rm -> ARM
                for p in range(Dn):
                    ie = node_in[v, p]
                    if ie >= n_edges or ie == e:
                        continue
                    ti = estep[ie]
                    if ti > t:
                        continue
                    assert ti <= t - 2, "node lag-1 should be impossible"
                    s = slot(ti * 64 + j)
                    atab[t, s, 128 + j] += 1.0        # m -> HP
        assert sidx[0].max() < 0 and sidx[1].max() < 0

        sidx_w = np.zeros((128, T, NIdx // 16), np.int16)
        for t in range(T):
            sidx_w[:, t, :] = _wrap_idx(sidx[t])
        atab_r = np.ascontiguousarray(
            atab.transpose(1, 0, 2).reshape(NIdx, T * 192))
        masks_r = np.broadcast_to(
            masks1.reshape(1, T * 128), (128, T * 128))

        # ---- per-step E slab: Ez[ws] | Eh[ws] | Er[wd], feature-major ----
        eidT = step_eid[:, trees]                     # [T, C]
        ws = wid[edge_src[eidT]]
        wd = wid[edge_dst[eidT]]
        escan = np.concatenate(
            [fmajor(Ez, ws), fmajor(Eh, ws), fmajor(Er, wd)], axis=3)
        escan = h16(escan.reshape(T * 128, 4 * 192))  # [T*128, 768]

        # ---- head Eu slab: col k*64+j (k=0 root, k=t+1 step t) ----
        widrow = np.zeros(NCOL, np.int64)
        widrow[:C] = wid[root_ids[trees]]
        for t in range(T):
            widrow[(t + 1) * 64:(t + 2) * 64] = wid[step_v[t, trees]]
        eug = Eu[widrow]                              # [NCOL, Hp]
        euslab = h16(eug.T.reshape(4, 128, NCOL).transpose(1, 0, 2)
                     .reshape(128, 4 * NCOL))

        # ---- tun / twn from tree_vec (host matmul) ----
        tvc = tvpad[trees]                            # [C, 64]
        def tvw(Wx):                                  # -> [128, 4, C] f16
            full = Wx.T @ tvc.T                       # [Hp, C]
            return h16(full.reshape(4, 128, C).transpose(1, 0, 2)
                       .reshape(128, 4 * C))
        tun_h = tvw(WuLx)
        twn_h = tvw(WwLx)

        # ---- q/p loss tables, row-major [128, 20] ----
        qmask = np.zeros((128, RC), np.float32)
        qtg = np.zeros((128, RC), np.float32)
        ptgt = np.zeros((128, RC), np.float32)
        pmask = np.zeros((128, RC), np.float32)
        for i in range(q_rows.shape[0]):
            g = int(q_rows[i])
            k, b = g // B, g % B
            if core * C <= b < (core + 1) * C:
                l = k * C + (b - core * C)
                qmask[l % 128, l // 128] = 1.0
                qtg[l % 128, l // 128] = float(q_tgt[i])
        for l in range(39 * C):
            k, j = l // C, l % C
            g = k * B + core * C + j
            ptgt[l % 128, l // 128] = float(p_tgt[g])
            pmask[l % 128, l // 128] = 1.0

        cores.append(dict(
            wz=wz_h, wh=wh_h, wu=wu_h, wuwh=wuwh_h, wwwh=wwwh_h,
            wo=wo_h, us=us_h, tun=tun_h, twn=twn_h,
            sidx=np.ascontiguousarray(sidx_w.reshape(128, T * (NIdx // 16))),
            atab=atab_r, masks=np.ascontiguousarray(masks_r).astype(np.float16),
            escan=escan, euslab=euslab,
            qtg=qtg, qmask=qmask, ptgt=ptgt, pmask=pmask,
            iota=np.broadcast_to(np.arange(V, dtype=np.float32), (128, V)).copy(),
        ))
    return cores


def _build_program():
    nc = bacc.Bacc("TRN2", debug=False)

    D = {}
    def di(name, shape, dt):
        D[name] = nc.dram_tensor(name, shape, dt, kind="ExternalInput")
        return D[name]

    for w in ["wz", "wh", "wu", "wuwh", "wwwh"]:
        di(w, [128, 4 * Hp], f16)
    di("wo", [128, 4 * V], f16)
    di("us", [128, 4], f16)
    di("tun", [128, 4 * C], f16)
    di("twn", [128, 4 * C], f16)
    di("sidx", [128, T * (NIdx // 16)], i16)
    di("atab", [NIdx, T * 192], f16)
    di("masks", [128, T * 128], f16)
    di("escan", [T * 128, 768], f16)
    di("euslab", [128, 4 * NCOL], f16)
    for x in ["qtg", "qmask", "ptgt", "pmask"]:
        di(x, [128, RC], f32)
    di("iota", [128, V], f32)
    out_d = nc.dram_tensor("out", [1, 8], f32, kind="ExternalOutput")

    mlog = nc.dram_tensor("mlog", [LOG_ROWS, 2 * Hp], f16, kind="Internal")
    pl_dram = nc.dram_tensor("pl_scratch", [1, NCOL], f32, kind="Internal")

    with tile.TileContext(nc) as tc:
        with tc.tile_pool(name="const", bufs=1) as cp:
            nc.gpsimd.load_library(_mlp_lib)
            rg128 = nc.gpsimd.to_reg(128)
            # ---- load constants ----
            def ld(name, shape, dt):
                t_ = cp.tile(shape, dt, tag=name)
                nc.sync.dma_start(out=t_[:], in_=D[name][:].rearrange(
                    "p (a b) -> p a b", a=shape[1]) if len(shape) == 3 else D[name][:])
                return t_
            wz = ld("wz", [128, 4, Hp], f16)
            wh = ld("wh", [128, 4, Hp], f16)
            wu = ld("wu", [128, 4, Hp], f16)
            wuwh = ld("wuwh", [128, 4, Hp], f16)
            wwwh = ld("wwwh", [128, 4, Hp], f16)
            wo = ld("wo", [128, 4, V], f16)
            us = ld("us", [128, 4], f16)
            tun = ld("tun", [128, 4, C], f16)
            twn = ld("twn", [128, 4, C], f16)
            sidx = ld("sidx", [128, T, NIdx // 16], i16)
            atab = ld("atab", [NIdx, T, 192], f16)
            masks = ld("masks", [128, T, 128], f16)
            qtg = ld("qtg", [128, RC], f32)
            qmask = ld("qmask", [128, RC], f32)
            ptgt = ld("ptgt", [128, RC], f32)
            pmask = ld("pmask", [128, RC], f32)
            iota_f = ld("iota", [128, V], f32)

            ident = cp.tile([128, 128], f16)
            from concourse.masks import make_identity
            make_identity(nc, ident[:])

            # h slab (feature-major), zeroed (roots + pad cols)
            hslab = cp.tile([128, 4, NCOL], f16)
            nc.vector.memset(hslab[:], 0.0)

            # ---------------- scan ----------------
            with tc.tile_pool(name="ep", bufs=4) as epp, \
                 tc.tile_pool(name="gp", bufs=3) as gpp, \
                 tc.tile_pool(name="wk", bufs=2) as wkp, \
                 tc.tile_pool(name="mrm", bufs=3) as mrmp, \
                 tc.tile_pool(name="shp", bufs=2, space="PSUM") as shps, \
                 tc.tile_pool(name="zhp", bufs=1, space="PSUM") as zhps, \
                 tc.tile_pool(name="rp", bufs=1, space="PSUM") as rps, \
                 tc.tile_pool(name="tpp", bufs=2, space="PSUM") as tpps:
                # pre-zero the gather pool slots: unfilled partitions are
                # multiplied by zero A-columns, and stale SBUF bytes can
                # decode as NaN/Inf (0*NaN = NaN poisons the routing matmuls)
                for _ in range(3):
                    gz = gpp.tile([128, 1, 1024], f16, tag="G")
                    nc.vector.memset(gz[:], 0.0)
                M_prev = None
                for t in range(T):
                    Et = epp.tile([128, 4, 192], f16, tag="Et")
                    nc.sync.dma_start(
                        out=Et[:],
                        in_=D["escan"][t * 128:(t + 1) * 128, :]
                        .rearrange("p (c n) -> p c n", c=4))

                    SHP = None
                    if t >= 2:
                        G = gpp.tile([128, 1, 1024], f16, tag="G")
                        nc.gpsimd.dma_gather(
                            G[:], mlog[0:(t - 1) * 64, :], sidx[:, t, :],
                            NIdx, rg128, 1024, transpose=False)
                        SHP = shps.tile([128, 4, 192], f32, space="PSUM",
                                        tag="SHP")
                        for m in range(4):
                            nc.tensor.matmul(
                                SHP[:, m, 0:64],
                                lhsT=G[:, 0, bass.ts(m, 128)],
                                rhs=atab[:, t, 0:64], start=True, stop=True)
                            nc.tensor.matmul(
                                SHP[:, m, 128:192],
                                lhsT=G[:, 0, bass.ts(m, 128)],
                                rhs=atab[:, t, 128:192], start=True, stop=True)
                        for m in range(4):
                            nc.tensor.matmul(
                                SHP[:, m, 64:128],
                                lhsT=G[:, 0, 512 + m * 128:512 + (m + 1) * 128],
                                rhs=atab[:, t, 64:128], start=True, stop=True)

                    # S assembly (f16, SBUF): lag-1 masked + gathered sums
                    S = None
                    if t >= 1:
                        S = wkp.tile([128, 4, 128], f16, tag="S")
                        mb = masks[:, t, :].rearrange(
                            "p (o n) -> p o n", o=1).to_broadcast([128, 4, 128])
                        if t == 1:
                            nc.vector.tensor_mul(S[:], M_prev[:], mb)
                        else:
                            lg = wkp.tile([128, 4, 128], f16, tag="lg")
                            nc.vector.tensor_mul(lg[:], M_prev[:], mb)
                            nc.vector.tensor_add(
                                S[:], SHP[:, :, 0:128], lg[:])

                    # z/h gates: E first (accumulation base), then W matmuls
                    zh_ps = zhps.tile([128, 8, 64], f32, space="PSUM", tag="zh")
                    for c in range(8):
                        nc.tensor.matmul(
                            zh_ps[:, c, :], lhsT=ident[:],
                            rhs=Et[:, c % 4, (c // 4) * 64:(c // 4) * 64 + 64],
                            start=(c == 0), stop=(t == 0 and c == 7))
                    if t >= 1:
                        for m in range(4):
                            for k in range(4):
                                nc.tensor.matmul(
                                    zh_ps[:, m, :],
                                    lhsT=wz[:, k, bass.ts(m, 128)],
                                    rhs=S[:, k, 0:64],
                                    start=False, stop=(k == 3))
                            for k in range(4):
                                nc.tensor.matmul(
                                    zh_ps[:, 4 + m, :],
                                    lhsT=wh[:, k, bass.ts(m, 128)],
                                    rhs=S[:, k, 64:128],
                                    start=False, stop=(k == 3))
                    z_t = wkp.tile([128, 4, 64], f16, tag="z")
                    nc.scalar.activation(z_t[:], zh_ps[:, 0:4, :], AF.Sigmoid)
                    th_t = wkp.tile([128, 4, 64], f16, tag="th")
                    nc.scalar.activation(th_t[:], zh_ps[:, 4:8, :], AF.Tanh)

                    Mt = mrmp.tile([128, 4, 128], f16, tag="M")
                    if t == 0:
                        nc.vector.tensor_mul(Mt[:, :, 0:64], z_t[:], th_t[:])
                    else:
                        d_t = wkp.tile([128, 4, 64], f16, tag="d")
                        nc.vector.tensor_sub(d_t[:], th_t[:], S[:, :, 0:64])
                        nc.vector.tensor_mul(d_t[:], z_t[:], d_t[:])
                        nc.vector.tensor_add(
                            Mt[:, :, 0:64], d_t[:], S[:, :, 0:64])

                    # r gate: Er base + Ur @ m_new
                    r_ps = rps.tile([128, 4, 64], f32, space="PSUM", tag="r")
                    for c in range(4):
                        nc.tensor.matmul(
                            r_ps[:, c, :], lhsT=ident[:],
                            rhs=Et[:, c, 128:192], start=(c == 0), stop=False)
                    for m in range(4):
                        for k in range(4):
                            nc.tensor.matmul(
                                r_ps[:, m, :],
                                lhsT=wu[:, k, bass.ts(m, 128)],
                                rhs=Mt[:, k, 0:64],
                                start=False, stop=(k == 3))
                    r_t = wkp.tile([128, 4, 64], f16, tag="rt")
                    nc.scalar.activation(r_t[:], r_ps[:], AF.Sigmoid)
                    nc.vector.tensor_mul(
                        Mt[:, :, 64:128], r_t[:], Mt[:, :, 0:64])

                    # node pull h -> hslab column block t+1
                    if t >= 2:
                        nc.vector.tensor_add(
                            hslab[:, :, bass.ts(t + 1, 64)],
                            SHP[:, :, 128:192], Mt[:, :, 0:64])
                    else:
                        nc.vector.tensor_copy(
                            hslab[:, :, bass.ts(t + 1, 64)], Mt[:, :, 0:64])

                    # transpose to tree-major, append to state log
                    tp_ = tpps.tile([64, 8, 128], f16, space="PSUM", tag="tp")
                    for c in range(4):
                        nc.tensor.transpose(
                            tp_[:, c, :], Mt[:, c, 0:64], ident[:])
                    for c in range(4):
                        nc.tensor.transpose(
                            tp_[:, 4 + c, :], Mt[:, c, 64:128], ident[:])
                    stw = wkp.tile([64, 1024], f16, tag="stw")
                    nc.any.tensor_copy(
                        stw[:, 0:512],
                        tp_[:, 0:4, :].rearrange("p a b -> p (a b)"))
                    nc.any.tensor_copy(
                        stw[:, 512:1024],
                        tp_[:, 4:8, :].rearrange("p a b -> p (a b)"))
                    nc.sync.dma_start(
                        out=mlog[t * 64:(t + 1) * 64, :], in_=stw[:])

                    M_prev = Mt

            # ---------------- heads ----------------
            acc = cp.tile([128, 8], f32)
            nc.vector.memset(acc[:], 0.0)
            pl_sb = cp.tile([1, NCOL], f32)
            tl_all = cp.tile([128, RC], f32)
            mxn_all = cp.tile([128, RC], f32)
            se_all = cp.tile([128, RC], f32)

            with tc.tile_pool(name="hd", bufs=1, space="PSUM") as hdp, \
                 tc.tile_pool(name="qlps", bufs=2, space="PSUM") as qlps, \
                 tc.tile_pool(name="hwk", bufs=2) as hwk, \
                 tc.tile_pool(name="eup", bufs=2) as eupp:
                for cc in range(5):
                    cs = slice(cc * 512, (cc + 1) * 512)
                    # ---- p chunk ----
                    eut = eupp.tile([128, 4, 512], f16, tag="eu")
                    nc.sync.dma_start(
                        out=eut[:],
                        in_=D["euslab"][:].rearrange(
                            "p (c n) -> p c n", c=4)[:, :, cs])
                    pp = hdp.tile([128, 4, 512], f32, space="PSUM", tag="big")
                    for m in range(4):
                        for k in range(4):
                            nc.tensor.matmul(pp[:, m, :],
                                             lhsT=wuwh[:, k, bass.ts(m, 128)],
                                             rhs=hslab[:, k, cs],
                                             start=(k == 0), stop=(k == 3))
                    ppre = hwk.tile([128, 4, 512], f16, tag="ppre")
                    nc.any.tensor_add(ppre[:], pp[:], eut[:])
                    nc.any.tensor_add(
                        ppre[:].rearrange("p c (i n) -> p c i n", i=8),
                        ppre[:].rearrange("p c (i n) -> p c i n", i=8),
                        tun[:].rearrange("p c (o n) -> p c o n", o=1)
                        .to_broadcast([128, 4, 8, C]))
                    nc.scalar.activation(ppre[:], ppre[:], AF.Relu)
                    pl_ps = qlps.tile([1, 512], f32, space="PSUM", tag="ql")
                    for k in range(4):
                        nc.tensor.matmul(pl_ps[:], lhsT=us[:, k:k + 1],
                                         rhs=ppre[:, k, :],
                                         start=(k == 0), stop=(k == 3))
                    nc.any.tensor_copy(pl_sb[0:1, cs], pl_ps[:])
                    # ---- q chunk (reductions overlap next chunk's matmuls) ----
                    qp = hdp.tile([128, 4, 512], f32, space="PSUM", tag="big")
                    for m in range(4):
                        for k in range(4):
                            nc.tensor.matmul(qp[:, m, :],
                                             lhsT=wwwh[:, k, bass.ts(m, 128)],
                                             rhs=hslab[:, k, cs],
                                             start=(k == 0), stop=(k == 3))
                    qpre = hwk.tile([128, 4, 512], f16, tag="qpre")
                    nc.any.tensor_add(
                        qpre[:].rearrange("p c (i n) -> p c i n", i=8),
                        qp[:].rearrange("p c (i n) -> p c i n", i=8),
                        twn[:].rearrange("p c (o n) -> p c o n", o=1)
                        .to_broadcast([128, 4, 8, C]))
                    nc.scalar.activation(qpre[:], qpre[:], AF.Relu)
                    for rr in range(4):
                        rc = cc * 4 + rr
                        ql = qlps.tile([128, V], f32, space="PSUM", tag="ql")
                        for k in range(4):
                            nc.tensor.matmul(ql[:, 0:512],
                                             lhsT=qpre[:, k, bass.ts(rr, 128)],
                                             rhs=wo[:, k, 0:512],
                                             start=(k == 0), stop=(k == 3))
                        for k in range(4):
                            nc.tensor.matmul(ql[:, 512:V],
                                             lhsT=qpre[:, k, bass.ts(rr, 128)],
                                             rhs=wo[:, k, 512:V],
                                             start=(k == 0), stop=(k == 3))
                        nc.vector.tensor_reduce(out=mxn_all[:, rc:rc + 1],
                                                in_=ql[:], op=ALU.max,
                                                axis=mybir.AxisListType.X,
                                                negate=True)
                        oh = hwk.tile([128, V], f32, tag="oh")
                        nc.vector.tensor_scalar(out=oh[:], in0=iota_f[:],
                                                scalar1=qtg[:, rc:rc + 1],
                                                scalar2=None, op0=ALU.is_equal)
                        ohp = hwk.tile([128, V], f32, tag="ohp")
                        nc.vector.tensor_tensor(out=ohp[:], in0=ql[:],
                                                in1=oh[:], op=ALU.mult)
                        nc.vector.tensor_reduce(out=tl_all[:, rc:rc + 1],
                                                in_=ohp[:], op=ALU.add,
                                                axis=mybir.AxisListType.X)
                        esc = hwk.tile([128, V], f32, tag="esc")
                        nc.scalar.activation(esc[:], ql[:], AF.Exp)
                        nc.vector.tensor_reduce(out=se_all[:, rc:rc + 1],
                                                in_=esc[:], op=ALU.add,
                                                axis=mybir.AxisListType.X)

            # ---- final reductions ----
                fin = cp.tile([128, RC], f32)
                # lse = ln(se); qterm = (lse - tl)*qmask summed
                nc.scalar.activation(fin[:], se_all[:], AF.Ln)
                nc.any.tensor_sub(fin[:], fin[:], tl_all[:])
                scr = cp.tile([128, RC], f32)
                nc.vector.tensor_tensor(out=scr[:], in0=fin[:], in1=qmask[:],
                                        op=ALU.mult)
                nc.vector.tensor_reduce(out=acc[:, 0:1], in_=scr[:], op=ALU.add,
                                        axis=mybir.AxisListType.X)
                # q match: tl + mxn == 0
                nc.any.tensor_add(fin[:], tl_all[:], mxn_all[:])
                nc.vector.tensor_scalar(out=fin[:], in0=fin[:], scalar1=0.0,
                                        scalar2=None, op0=ALU.is_equal)
                nc.vector.tensor_tensor(out=scr[:], in0=fin[:], in1=qmask[:],
                                        op=ALU.mult)
                nc.vector.tensor_reduce(out=acc[:, 2:3], in_=scr[:], op=ALU.add,
                                        axis=mybir.AxisListType.X)

                # p head: reshape pl [1, 2560] -> [128, 20] via DRAM round-trip
                nc.sync.dma_start(out=pl_dram[:], in_=pl_sb[:])
                pl_rm = cp.tile([128, RC], f32)
                nc.sync.dma_start(
                    out=pl_rm[:],
                    in_=pl_dram[0:1, :].rearrange("o (rc p) -> (o p) rc", p=128))
                # softplus(x) = relu(x) + ln(1 + exp(-|x|))
                ab = cp.tile([128, RC], f32)
                nc.scalar.activation(ab[:], pl_rm[:], AF.Abs)
                nc.scalar.activation(ab[:], ab[:], AF.Exp, scale=-1.0)
                nc.scalar.activation(ab[:], ab[:], AF.Ln, bias=1.0)
                rl = cp.tile([128, RC], f32)
                nc.scalar.activation(rl[:], pl_rm[:], AF.Relu)
                nc.any.tensor_add(ab[:], ab[:], rl[:])
                nc.vector.tensor_tensor(out=scr[:], in0=ab[:], in1=pmask[:],
                                        op=ALU.mult)
                nc.vector.tensor_reduce(out=acc[:, 1:2], in_=scr[:], op=ALU.add,
                                        axis=mybir.AxisListType.X)
                nc.vector.tensor_tensor(out=scr[:], in0=pl_rm[:], in1=ptgt[:],
                                        op=ALU.mult)
                nc.vector.tensor_reduce(out=acc[:, 4:5], in_=scr[:], op=ALU.add,
                                        axis=mybir.AxisListType.X)
                # p match: (pl > 0) == ptgt
                gt = cp.tile([128, RC], f32)
                nc.vector.tensor_scalar(out=gt[:], in0=pl_rm[:], scalar1=0.0,
                                        scalar2=None, op0=ALU.is_gt)
                nc.vector.tensor_tensor(out=gt[:], in0=gt[:], in1=ptgt[:],
                                        op=ALU.is_equal)
                nc.vector.tensor_tensor(out=scr[:], in0=gt[:], in1=pmask[:],
                                        op=ALU.mult)
                nc.vector.tensor_reduce(out=acc[:, 3:4], in_=scr[:], op=ALU.add,
                                        axis=mybir.AxisListType.X)

                with tc.tile_pool(name="fps", bufs=1, space="PSUM") as fps:
                    ones32 = cp.tile([128, 1], f32)
                    nc.vector.memset(ones32[:], 1.0)
                    fin_ps = fps.tile([1, 8], f32, space="PSUM")
                    nc.tensor.matmul(fin_ps[:], lhsT=ones32[:], rhs=acc[:],
                                     start=True, stop=True)
                    fin_sb = cp.tile([1, 8], f32)
                    nc.vector.tensor_copy(fin_sb[:], fin_ps[:])
                    nc.sync.dma_start(out=out_d[:], in_=fin_sb[:])

    nc.compile()
    return nc


_NC_CACHE = None
LAST_EXEC_NS = None
LAST_RES = None


def kernel(**inputs):
    global _NC_CACHE
    cores = _host_prep(inputs)
    if _NC_CACHE is None:
        _NC_CACHE = _build_program()
    nc = _NC_CACHE
    in_maps = [{k: np.ascontiguousarray(v) for k, v in cores[c].items()}
               for c in range(NC)]
    trace = os.environ.get("KERNEL_TRACE", "0") == "1"
    res = run_bass_kernel_spmd(nc, in_maps, core_ids=list(range(NC)),
                               trace=trace)
    global LAST_EXEC_NS, LAST_RES
    LAST_EXEC_NS = getattr(res, "exec_time_ns", None)
    LAST_RES = res
    total = np.zeros(8, np.float64)
    for r in res.results:
        total += np.asarray(r["out"], np.float64).reshape(-1)
    q_loss = total[0] / B
    p_loss = (total[1] - total[4]) / B
    q_acc = total[2] / 10240.0
    p_acc = total[3] / (39 * B)
    return np.array([q_loss, p_loss, q_acc, p_acc], np.float32)


if __name__ == "__main__":
    pass
